# revision 1
# baseline (speedup 1.0000x reference)
"""Bass program builders + host orchestration for the CustomGAT kernel.

Three SPMD launches on 8 cores:
  L1: pano GAT layer 0   (in: x_panoT, edges pp) -> p0' slices
  L2: pano GAT layer 1   (in: p0'T, edges pp)    -> p1' slices
  L3: translate conv + NullModel + closing MLP   -> final [1, 20*128] slices

Edge phase per core: dst-sorted edges packed in 128-edge tiles that never
cross 128-dst chunks; tiles grouped per (chunk, src-bank), bank-major
stream; per-tile selector matmul accumulates [wmsg|exp] into a psum run;
runs accumulate into an SBUF accumulator; finalize divides + bias.
Gathers via gpsimd.dma_gather (int16 idx, 512B rows).
"""
import numpy as np

import concourse.bass as bass
import concourse.bacc as bacc
import concourse.mybir as mybir
from concourse.tile import TileContext
from concourse.vector_clock import ScopedClock
from concourse import bass_utils

F32 = mybir.dt.float32
I16 = mybir.dt.int16
AF = mybir.ActivationFunctionType
OP = mybir.AluOpType

P = 128
N_CORES = 8
BANK = 32768
G = 24                 # max tiles per gather batch
S = 4                  # tiles per compute subgroup


# ---------------------------------------------------------------- drain patch
def _patched_drain_and_barrier(self, tick_clock, wait_clock):
    victim = self.nc.sync.nop(nofuse=True)
    wait_clock.add_sem_waits(victim.ins, ScopedClock({None: tick_clock.global_clock}))
    si = victim.ins.sync_info
    waits = list(si.on_wait) if si is not None and si.on_wait else []
    if si is not None and len(waits) > 1:
        si.on_wait = waits[:1]
        for w in waits[1:]:
            extra = self.nc.sync.nop(nofuse=True)
            esi = extra.ins.sync_info
            if esi is None:
                extra.ins.sync_info = mybir.SyncInfo(on_wait=[w], on_update=[])
            else:
                esi.on_wait = [w]
    self.nc.sync.drain()
    self.nc.all_engine_barrier()
    popped = self.nc._tile_sem_poison_stack.pop()
    assert popped is self._sem_poison
    self.nc.clear_and_free_semaphores(list(self.sems.allocated().values()))
    self.nc.all_engine_barrier()


TileContext._drain_and_barrier = _patched_drain_and_barrier


# ---------------------------------------------------------------- host: plan
class Plan:
    __slots__ = ('n_chunks', 'n_banks', 'tiles', 'batches', 'runs', 'T',
                 'acc_mode', 'n_table_rows', 'final_acc')

    def __init__(self, **kw):
        for k, v in kw.items():
            setattr(self, k, v)


def build_plan_and_streams(src, dst, n_chunks_per_core, n_table_rows):
    """Returns (plan, per_core_streams)."""
    src = np.asarray(src, np.int64)
    dst = np.asarray(dst, np.int64)
    order = np.argsort(dst, kind='stable')
    s_src = src[order]
    s_dst = dst[order]
    n_banks = int(np.ceil(n_table_rows / BANK))
    core_span = n_chunks_per_core * P

    counts = np.zeros((N_CORES, n_chunks_per_core, n_banks), np.int64)
    lists = [[None] * n_chunks_per_core for _ in range(N_CORES)]
    for c in range(N_CORES):
        lo = np.searchsorted(s_dst, c * core_span, side='left')
        hi = np.searchsorted(s_dst, (c + 1) * core_span, side='left')
        cs, cd = s_src[lo:hi], s_dst[lo:hi]
        k_arr = (cd - c * core_span) // P
        b_arr = cs // BANK
        key = k_arr * n_banks + b_arr
        o2 = np.argsort(key, kind='stable')
        cs, cd, key = cs[o2], cd[o2], key[o2]
        bounds = np.searchsorted(key, np.arange(n_chunks_per_core * n_banks + 1))
        for k in range(n_chunks_per_core):
            per_bank = []
            for m in range(n_banks):
                i0, i1 = bounds[k * n_banks + m], bounds[k * n_banks + m + 1]
                per_bank.append((cs[i0:i1], cd[i0:i1]))
                counts[c, k, m] = i1 - i0
            lists[c][k] = per_bank

    tiles_km = np.ceil(counts / P).astype(np.int64).max(axis=0)
    empty = tiles_km.sum(axis=1) == 0
    tiles_km[empty, 0] = 1

    tiles, runs, acc_mode = [], [], []
    seen = set()
    for m in range(n_banks):
        for k in range(n_chunks_per_core):
            tk = int(tiles_km[k, m])
            if tk == 0:
                continue
            for i in range(tk):
                tiles.append((k, m))
                runs.append((i == 0, i == tk - 1))
                if i == tk - 1:
                    acc_mode.append('copy' if k not in seen else 'add')
                else:
                    acc_mode.append(None)
            seen.add(k)
    T = len(tiles)

    batches = []
    t = 0
    while t < T:
        m = tiles[t][1]
        n = 1
        while t + n < T and tiles[t + n][1] == m and n < G:
            n += 1
        batches.append((m, t, n))
        t += n

    final_acc = [False] * T
    last_end = {}
    for t in range(T):
        if runs[t][1]:
            last_end[tiles[t][0]] = t
    for k, t in last_end.items():
        final_acc[t] = True
    plan = Plan(n_chunks=n_chunks_per_core, n_banks=n_banks, tiles=tiles,
                batches=batches, runs=runs, T=T, acc_mode=acc_mode,
                n_table_rows=n_table_rows, final_acc=final_acc)

    streams = []
    for c in range(N_CORES):
        esrc = np.zeros((T, P), np.int64)
        hrloc = np.zeros((T, P), np.int64)
        dstloc = np.full((T, P), -1.0, np.float32)
        t = 0
        for m in range(n_banks):
            for k in range(n_chunks_per_core):
                tk = int(tiles_km[k, m])
                if tk == 0:
                    continue
                es, ed = lists[c][k][m]
                ne = len(es)
                fe = np.zeros(tk * P, np.int64)
                fh = np.zeros(tk * P, np.int64)
                fd = np.full(tk * P, -1.0, np.float32)
                fe[:ne] = es
                fh[:ne] = ed - c * core_span
                fd[:ne] = (ed - (c * core_span + k * P)).astype(np.float32)
                esrc[t:t + tk] = fe.reshape(tk, P)
                hrloc[t:t + tk] = fh.reshape(tk, P)
                dstloc[t:t + tk] = fd.reshape(tk, P)
                t += tk
        streams.append(dict(esrc=esrc, hrloc=hrloc, dstloc=dstloc))
    return plan, streams


def wrap_idx16(flat_idx):
    """[T,128] -> [128, T*8] int16 dma_gather layout (16-wrap, x8 replicated)."""
    n = flat_idx.size
    x = flat_idx.reshape(n)
    w = np.zeros((16, n // 16), np.int16)
    pos = np.arange(n)
    w[pos % 16, pos // 16] = x.astype(np.int16)
    return np.tile(w, (8, 1))


def make_stream_inputs(stream, lamL, lamR):
    esrc, hrloc, dstloc = stream['esrc'], stream['hrloc'], stream['dstloc']
    hl_idx = wrap_idx16(esrc % BANK)
    hr_idx = wrap_idx16(hrloc)
    dstlocT = np.ascontiguousarray(dstloc.T)
    L = (lamL[esrc] + lamR[hrloc]).astype(np.float32)       # [T,128,2]
    LT = np.ascontiguousarray(L.transpose(1, 0, 2).reshape(P, -1))
    return dict(hl_idx=hl_idx, hr_idx=hr_idx, dstlocT=dstlocT, LT=LT)


# ---------------------------------------------------------- conv transforms
def conv_transform(Wl, bl, Wr, br, att, b):
    H, C = att.shape
    a = np.asarray(att, np.float64).reshape(-1)
    perm, widths = [], []
    for h in range(H):
        cols = np.arange(h * C, (h + 1) * C)
        pos = cols[a[cols] >= 0]
        neg = cols[a[cols] < 0]
        widths.append(len(pos))
        perm.extend(pos.tolist())
        perm.extend(neg.tolist())
    perm = np.array(perm, np.int64)
    A = np.maximum(0.8 * np.abs(a[perm]), 1e-12)

    def scale_cols(W, bvec):
        W = np.asarray(W, np.float64)
        bvec = np.asarray(bvec, np.float64)
        return ((W[:, perm] * A[None, :]).astype(np.float32),
                (bvec[perm] * A).astype(np.float32))

    Wl_s, bl_s = scale_cols(Wl, bl)
    Wr_s, br_s = scale_cols(Wr, br)
    # lambda projectors (true-space): lam = x @ Wlam + blam, per head, x0.2
    Wlam_l = np.stack([0.2 * (np.asarray(Wl, np.float64)[:, h * C:(h + 1) * C]
                              @ a[h * C:(h + 1) * C]) for h in range(H)], 1)
    blam_l = np.array([0.2 * (np.asarray(bl, np.float64)[h * C:(h + 1) * C]
                              @ a[h * C:(h + 1) * C]) for h in range(H)])
    Wlam_r = np.stack([0.2 * (np.asarray(Wr, np.float64)[:, h * C:(h + 1) * C]
                              @ a[h * C:(h + 1) * C]) for h in range(H)], 1)
    blam_r = np.array([0.2 * (np.asarray(br, np.float64)[h * C:(h + 1) * C]
                              @ a[h * C:(h + 1) * C]) for h in range(H)])
    bprime = (np.asarray(b, np.float64)[perm] * A).astype(np.float32)
    return dict(perm=perm, A=A, widths=widths, Wl=Wl_s, bl=bl_s, Wr=Wr_s, br=br_s,
                Wlam_l=Wlam_l, blam_l=blam_l, Wlam_r=Wlam_r, blam_r=blam_r,
                bprime=bprime)


def input_fixup(W, perm, A):
    """Row-fixup so W consumes stored p' (scaled+permuted) instead of p."""
    W = np.asarray(W, np.float64)
    return (W[perm, :] / A[:, None]).astype(np.float32)


def rep(v):
    """Replicate row vector across 128 partitions."""
    v = np.asarray(v, np.float32).reshape(1, -1)
    return np.ascontiguousarray(np.repeat(v, P, 0))


COLS_CONST = np.ascontiguousarray(
    np.repeat(np.arange(P, dtype=np.float32)[None, :], P, 0))


# ------------------------------------------------------------ device pieces
def _edge_phase(nc, tc, plan, hl_tabs, hr_table, hl_idx, hr_idx, dstlocT, LT,
                cols_sb, acc, widths, bprep_sb, out_cb):
    w0, w1 = widths
    ranges = [(0, w0), (64, 64 + w1), (w0, 64), (64 + w1, 128)]  # RP0,RP1,RN0,RN1
    with (
        tc.tile_pool(name='eidx', bufs=3) as idx_pool,
        tc.tile_pool(name='emsg', bufs=3) as msg_pool,
        tc.tile_pool(name='esg', bufs=4) as sg_pool,
        tc.tile_pool(name='erp', bufs=6, space='PSUM') as run_psum_pool,
        tc.tile_pool(name='fin', bufs=4) as fin_pool,
    ):
        cur_psum = [None]
        MB = 4 * G
        megas = []
        for (bank, t0, nt) in plan.batches:
            if megas and megas[-1][0] + megas[-1][1] == t0 and \
                    megas[-1][1] + nt <= MB:
                megas[-1] = (megas[-1][0], megas[-1][1] + nt,
                             megas[-1][2] + [(bank, t0, nt)])
            else:
                megas.append((t0, nt, [(bank, t0, nt)]))
        for (tm0, tmn, bl) in megas:
            hli = idx_pool.tile([P, MB * 8], I16, tag='hli')
            hri = idx_pool.tile([P, MB * 8], I16, tag='hri')
            dlo = idx_pool.tile([P, MB], F32, tag='dlo')
            ltt = idx_pool.tile([P, MB * 2], F32, tag='ltt')
            nc.sync.dma_start(out=hli[:, :tmn * 8],
                              in_=hl_idx[:, tm0 * 8:(tm0 + tmn) * 8])
            nc.sync.dma_start(out=hri[:, :tmn * 8],
                              in_=hr_idx[:, tm0 * 8:(tm0 + tmn) * 8])
            nc.sync.dma_start(out=dlo[:, :tmn], in_=dstlocT[:, tm0:tm0 + tmn])
            nc.sync.dma_start(out=ltt[:, :tmn * 2],
                              in_=LT[:, tm0 * 2:(tm0 + tmn) * 2])
            for (bank, t0, nt) in bl:
                r0 = t0 - tm0
                msg = msg_pool.tile([P, G * P], F32, tag='msg')
                hrg = msg_pool.tile([P, G * P], F32, tag='hrg')
                nc.gpsimd.dma_gather(
                    out_ap=msg[:, :nt * P].rearrange("p (t d) -> p t d", d=P),
                    in_ap=hl_tabs[bank][:, :],
                    idxs_ap=hli[:, r0 * 8:(r0 + nt) * 8],
                    num_idxs=nt * P, num_idxs_reg=nt * P,
                    elem_size=P, single_packet=False)
                nc.gpsimd.dma_gather(
                    out_ap=hrg[:, :nt * P].rearrange("p (t d) -> p t d", d=P),
                    in_ap=hr_table[:, :],
                    idxs_ap=hri[:, r0 * 8:(r0 + nt) * 8],
                    num_idxs=nt * P, num_idxs_reg=nt * P,
                    elem_size=P, single_packet=False)
                for s0 in range(0, nt, S):
                    ns = min(S, nt - s0)
                    q0 = r0 + s0
                    sel = sg_pool.tile([P, S * P], F32, tag='sel')
                    tsb = sg_pool.tile([P, S * P], F32, tag='tsb')
                    usb = sg_pool.tile([P, S * P], F32, tag='usb')
                    rhs = sg_pool.tile([P, S * 130], F32, tag='rhs')
                    red = sg_pool.tile([P, S * 4], F32, tag='red')
                    ssb = sg_pool.tile([P, S * 2], F32, tag='ssb')
                    m_sl = msg[:, s0 * P:(s0 + ns) * P]
                    h_sl = hrg[:, s0 * P:(s0 + ns) * P]
                    nc.vector.tensor_tensor(
                        out=sel[:, :ns * P].rearrange("p (j c) -> p j c", c=P),
                        in0=cols_sb[:].rearrange("p (o c) -> p o c", o=1)
                        .to_broadcast([P, ns, P]),
                        in1=dlo[:, q0:q0 + ns].rearrange("p (j o) -> p j o", o=1)
                        .to_broadcast([P, ns, P]),
                        op=OP.is_equal)
                    nc.vector.tensor_tensor(out=tsb[:, :ns * P], in0=m_sl,
                                            in1=h_sl, op=OP.add)
                    nc.scalar.activation(out=usb[:, :ns * P], in_=tsb[:, :ns * P],
                                         func=AF.Relu)
                    uv = usb[:, :ns * P].rearrange("p (j c) -> p j c", c=P)
                    rv = red[:, :ns * 4].rearrange("p (j f) -> p j f", f=4)
                    for ri, (c0, c1) in enumerate(ranges):
                        nc.vector.tensor_reduce(
                            out=rv[:, :, ri:ri + 1],
                            in_=uv[:, :, c0:c1],
                            axis=mybir.AxisListType.X, op=OP.add)
                    sv = ssb[:, :ns * 2].rearrange("p (j h) -> p j h", h=2)
                    lv = ltt[:, q0 * 2:(q0 + ns) * 2].rearrange(
                        "p (j h) -> p j h", h=2)
                    nc.vector.tensor_tensor(out=sv, in0=lv, in1=rv[:, :, 0:2],
                                            op=OP.add)
                    nc.vector.tensor_tensor(out=sv, in0=sv, in1=rv[:, :, 2:4],
                                            op=OP.subtract)
                    rview = rhs[:, :ns * 130].rearrange("p (j c) -> p j c", c=130)
                    nc.scalar.activation(out=rview[:, :, 128:130], in_=sv,
                                         func=AF.Exp)
                    nc.gpsimd.tensor_tensor(
                        out=rview[:, :, 0:128].rearrange(
                            "p j (h c) -> p j h c", c=64),
                        in0=m_sl.rearrange("p (j h c) -> p j h c", h=2, c=64),
                        in1=rview[:, :, 128:130].rearrange(
                            "p j (h o) -> p j h o", o=1)
                        .to_broadcast([P, ns, 2, 64]),
                        op=OP.mult)
                    for j in range(ns):
                        t_idx = t0 + s0 + j
                        run_start, run_end = plan.runs[t_idx]
                        if run_start:
                            cur_psum[0] = run_psum_pool.tile(
                                [P, 130], F32, tag='runp', name='runp')
                        nc.tensor.matmul(
                            out=cur_psum[0][:],
                            lhsT=sel[:, j * P:(j + 1) * P],
                            rhs=rhs[:, j * 130:(j + 1) * 130],
                            start=run_start, stop=run_end)
                        if run_end:
                            k = plan.tiles[t_idx][0]
                            a_sl = acc[:, k * 130:(k + 1) * 130]
                            if plan.acc_mode[t_idx] == 'copy':
                                nc.scalar.activation(out=a_sl,
                                                     in_=cur_psum[0][:],
                                                     func=AF.Copy)
                            else:
                                nc.vector.tensor_tensor(out=a_sl, in0=a_sl,
                                                        in1=cur_psum[0][:],
                                                        op=OP.add)
                            if plan.final_acc[t_idx]:
                                _finalize_chunk(nc, fin_pool, acc, k,
                                                bprep_sb, out_cb)


def _finalize_chunk(nc, fin_pool, acc, k, bprep_sb, out_cb):
    dadj = fin_pool.tile([P, 2], F32, tag='dadj', name='dadj')
    rec = fin_pool.tile([P, 2], F32, tag='rec', name='rec')
    res = fin_pool.tile([P, P], F32, tag='res', name='res')
    nc.vector.tensor_scalar_add(
        out=dadj[:], in0=acc[:, k * 130 + 128:k * 130 + 130], scalar1=1e-16)
    nc.vector.reciprocal(out=rec[:], in_=dadj[:])
    nc.vector.tensor_tensor(
        out=res[:].rearrange("p (h c) -> p h c", c=64),
        in0=acc[:, k * 130:k * 130 + 128].rearrange("p (h c) -> p h c", c=64),
        in1=rec[:].rearrange("p (h o) -> p h o", o=1).to_broadcast([P, 2, 64]),
        op=OP.mult)
    nc.vector.tensor_tensor(out=res[:], in0=res[:], in1=bprep_sb[:], op=OP.add)
    out_cb(k, res)


def _finalize(nc, tc, plan, acc, bprep_sb, out_cb):
    with tc.tile_pool(name='fin', bufs=4) as fin_pool:
        for k in range(plan.n_chunks):
            dadj = fin_pool.tile([P, 2], F32, tag='dadj')
            rec = fin_pool.tile([P, 2], F32, tag='rec')
            res = fin_pool.tile([P, P], F32, tag='res')
            nc.vector.tensor_scalar_add(
                out=dadj[:], in0=acc[:, k * 130 + 128:k * 130 + 130],
                scalar1=1e-16)
            nc.vector.reciprocal(out=rec[:], in_=dadj[:])
            nc.vector.tensor_tensor(
                out=res[:].rearrange("p (h c) -> p h c", c=64),
                in0=acc[:, k * 130:k * 130 + 128].rearrange(
                    "p (h c) -> p h c", c=64),
                in1=rec[:].rearrange("p (h o) -> p h o", o=1)
                .to_broadcast([P, 2, 64]),
                op=OP.mult)
            nc.vector.tensor_tensor(out=res[:], in0=res[:], in1=bprep_sb[:],
                                    op=OP.add)
            out_cb(k, res)


def build_pano_layer(plan, D_in, widths):
    nc = bacc.Bacc("TRN2", target_bir_lowering=False, debug=False,
                   num_devices=N_CORES)
    NK = plan.n_chunks
    NROWS = plan.n_table_rows
    T = plan.T
    xT = nc.dram_tensor('xT', [D_in, NROWS], F32, kind='ExternalInput')
    xTs = nc.dram_tensor('xTs', [D_in, NK * P], F32, kind='ExternalInput')
    Wl = nc.dram_tensor('Wl', [D_in, P], F32, kind='ExternalInput')
    Wr = nc.dram_tensor('Wr', [D_in, P], F32, kind='ExternalInput')
    blrep = nc.dram_tensor('blrep', [P, P], F32, kind='ExternalInput')
    brrep = nc.dram_tensor('brrep', [P, P], F32, kind='ExternalInput')
    bprep = nc.dram_tensor('bprep', [P, P], F32, kind='ExternalInput')
    colsc = nc.dram_tensor('colsc', [P, P], F32, kind='ExternalInput')
    hl_idx = nc.dram_tensor('hl_idx', [P, T * 8], I16, kind='ExternalInput')
    hr_idx = nc.dram_tensor('hr_idx', [P, T * 8], I16, kind='ExternalInput')
    dstlocT = nc.dram_tensor('dstlocT', [P, T], F32, kind='ExternalInput')
    LT = nc.dram_tensor('LT', [P, T * 2], F32, kind='ExternalInput')
    p_out = nc.dram_tensor('p_out', [NK * P, P], F32, kind='ExternalOutput')
    hl_tabs = [nc.dram_tensor(f'hl_table{m}',
                              [min(BANK, NROWS - m * BANK), P], F32,
                              kind='Internal')
               for m in range(plan.n_banks)]
    hr_table = nc.dram_tensor('hr_table', [NK * P, P], F32, kind='Internal')

    with TileContext(nc) as tc:
        with tc.tile_pool(name='const', bufs=1) as cpool:
            Wl_sb = cpool.tile([D_in, P], F32)
            Wr_sb = cpool.tile([D_in, P], F32)
            blrep_sb = cpool.tile([P, P], F32, tag='blrep')
            brrep_sb = cpool.tile([P, P], F32, tag='brrep')
            bprep_sb = cpool.tile([P, P], F32)
            cols_sb = cpool.tile([P, P], F32)
            acc = cpool.tile([P, NK * 130], F32)
            nc.sync.dma_start(out=Wl_sb[:], in_=Wl[:])
            nc.sync.dma_start(out=Wr_sb[:], in_=Wr[:])
            nc.sync.dma_start(out=blrep_sb[:], in_=blrep[:])
            nc.sync.dma_start(out=brrep_sb[:], in_=brrep[:])
            nc.sync.dma_start(out=bprep_sb[:], in_=bprep[:])
            nc.sync.dma_start(out=cols_sb[:], in_=colsc[:])

            with (
                tc.tile_pool(name='dps', bufs=3, space='PSUM') as psum_pool,
                tc.tile_pool(name='dstage', bufs=3) as stage_pool,
                tc.tile_pool(name='dxpage', bufs=3) as xpage_pool,
            ):
                _dense_table2(nc, tc, xTs, Wr_sb, hr_table, NK,
                              psum_pool, stage_pool, xpage_pool, brrep_sb)
                _dense_table2(nc, tc, xT, Wl_sb, hl_tabs, NROWS // P,
                              psum_pool, stage_pool, xpage_pool, blrep_sb)

            def emit(k, res):
                nc.sync.dma_start(out=p_out[k * P:(k + 1) * P, :], in_=res[:])
            _edge_phase(nc, tc, plan, hl_tabs, hr_table, hl_idx, hr_idx,
                        dstlocT, LT, cols_sb, acc, widths, bprep_sb, emit)
    nc.compile()
    return nc


def _dense_table2(nc, tc, xT, W_sb, table, n_tiles,
                  psum_pool, stage_pool, xpage_pool, brep_sb,
                  page_tiles=8):
    D = xT.shape[0]
    tabs = table if isinstance(table, list) else [table]
    if len(tabs) > 1:
        page_tiles = min(page_tiles, max(1, BANK // P))
    n_pages = (n_tiles + page_tiles - 1) // page_tiles
    for pg in range(n_pages):
        j0 = pg * page_tiles
        jn = min(page_tiles, n_tiles - j0)
        xp = xpage_pool.tile([D, page_tiles * P], F32, tag='xpage')
        nc.gpsimd.dma_start(out=xp[:, :jn * P], in_=xT[:, j0 * P:(j0 + jn) * P])
        stage = stage_pool.tile([P, page_tiles * P], F32, tag='stage')
        ps = psum_pool.tile([P, page_tiles * P], F32, tag='dps', name='dps')
        for j in range(jn):
            nc.tensor.matmul(out=ps[:, j * P:(j + 1) * P],
                             lhsT=xp[:, j * P:(j + 1) * P], rhs=W_sb[:],
                             start=True, stop=True)
        nc.vector.tensor_tensor(
            out=stage[:, :jn * P].rearrange("p (j c) -> p j c", c=P),
            in0=ps[:, :jn * P].rearrange("p (j c) -> p j c", c=P),
            in1=brep_sb[:].rearrange("p (o c) -> p o c", o=1)
            .to_broadcast([P, jn, P]),
            op=OP.add)
        r0 = j0 * P
        m = r0 // BANK
        lr = r0 - m * BANK
        nc.scalar.dma_start(
            out=tabs[m][lr:lr + jn * P, :].rearrange("(j p) c -> p j c", p=P),
            in_=stage[:, :jn * P].rearrange("p (j c) -> p j c", c=P))

def build_l3(plan, D_hl, widths):
    """Translate conv + NullModel + closing MLP. D_hl = 128 (p1' feats)."""
    nc = bacc.Bacc("TRN2", target_bir_lowering=False, debug=False,
                   num_devices=N_CORES)
    NK = plan.n_chunks           # 20
    NROWS = plan.n_table_rows    # 100352 (pano side)
    NFP = NK * P                 # 2560 local fp rows
    T = plan.T
    DF = 16
    xT = nc.dram_tensor('xT', [D_hl, NROWS], F32, kind='ExternalInput')      # p1'T
    fTs = nc.dram_tensor('fTs', [DF, NFP], F32, kind='ExternalInput')        # x_fpT slice
    Wl = nc.dram_tensor('Wl', [D_hl, P], F32, kind='ExternalInput')
    Wr = nc.dram_tensor('Wr', [DF, P], F32, kind='ExternalInput')
    blrep = nc.dram_tensor('blrep', [P, P], F32, kind='ExternalInput')
    brrep = nc.dram_tensor('brrep', [P, P], F32, kind='ExternalInput')
    bprep = nc.dram_tensor('bprep', [P, P], F32, kind='ExternalInput')
    colsc = nc.dram_tensor('colsc', [P, P], F32, kind='ExternalInput')
    ident = nc.dram_tensor('ident', [P, P], F32, kind='ExternalInput')
    hl_idx = nc.dram_tensor('hl_idx', [P, T * 8], I16, kind='ExternalInput')
    hr_idx = nc.dram_tensor('hr_idx', [P, T * 8], I16, kind='ExternalInput')
    dstlocT = nc.dram_tensor('dstlocT', [P, T], F32, kind='ExternalInput')
    LT = nc.dram_tensor('LT', [P, T * 2], F32, kind='ExternalInput')
    # MLP + NullModel weights
    mw1 = nc.dram_tensor('mw1', [P, 64], F32, kind='ExternalInput')   # input-fixed
    mb1 = nc.dram_tensor('mb1', [64, 1], F32, kind='ExternalInput')
    mw2 = nc.dram_tensor('mw2', [64, 64], F32, kind='ExternalInput')
    mb2 = nc.dram_tensor('mb2', [64, 1], F32, kind='ExternalInput')
    mw3 = nc.dram_tensor('mw3', [64, 1], F32, kind='ExternalInput')
    mb3 = nc.dram_tensor('mb3', [1, 1], F32, kind='ExternalInput')
    nsw = nc.dram_tensor('nsw', [DF, 64], F32, kind='ExternalInput')
    nsb = nc.dram_tensor('nsb', [64, 1], F32, kind='ExternalInput')
    nbw = nc.dram_tensor('nbw', [64, 64], F32, kind='ExternalInput')
    nbb = nc.dram_tensor('nbb', [64, 1], F32, kind='ExternalInput')
    ncw = nc.dram_tensor('ncw', [64, 1], F32, kind='ExternalInput')
    ncb = nc.dram_tensor('ncb', [1, 1], F32, kind='ExternalInput')
    nlw = nc.dram_tensor('nlw', [DF, 1], F32, kind='ExternalInput')
    nlb = nc.dram_tensor('nlb', [1, 1], F32, kind='ExternalInput')
    out = nc.dram_tensor('out', [1, NFP], F32, kind='ExternalOutput')
    hl_tabs = [nc.dram_tensor(f'hl_table{m}',
                              [min(BANK, NROWS - m * BANK), P], F32,
                              kind='Internal')
               for m in range(plan.n_banks)]
    hr_table = nc.dram_tensor('hr_table', [NFP, P], F32, kind='Internal')

    with TileContext(nc) as tc:
        with tc.tile_pool(name='const', bufs=1) as cpool:
            Wl_sb = cpool.tile([D_hl, P], F32)
            Wr_sb = cpool.tile([DF, P], F32)
            blrep_sb = cpool.tile([P, P], F32, tag='blrep')
            brrep_sb = cpool.tile([P, P], F32, tag='brrep')
            bprep_sb = cpool.tile([P, P], F32)
            cols_sb = cpool.tile([P, P], F32)
            id_sb = cpool.tile([P, P], F32)
            acc = cpool.tile([P, NK * 130], F32)
            fpT_sb = cpool.tile([P, NFP], F32)
            fT_sb = cpool.tile([DF, NFP], F32)
            sm = cpool.tile([P, 64 + 64 + 1 + 64 + 64 + 1 + 1], F32)  # packed small weights
            for dst_sb, src_d in ((Wl_sb, Wl), (Wr_sb, Wr),
                                  (bprep_sb, bprep), (cols_sb, colsc),
                                  (id_sb, ident), (fT_sb, fTs),
                                  (blrep_sb, blrep), (brrep_sb, brrep)):
                nc.sync.dma_start(out=dst_sb[:], in_=src_d[:])
            mw1_sb = cpool.tile([P, 64], F32)
            mw2_sb = cpool.tile([64, 64], F32)
            mw3_sb = cpool.tile([64, 1], F32)
            nsw_sb = cpool.tile([DF, 64], F32)
            nbw_sb = cpool.tile([64, 64], F32)
            ncw_sb = cpool.tile([64, 1], F32)
            nlw_sb = cpool.tile([DF, 1], F32)
            mb1_sb = cpool.tile([64, 1], F32)
            mb2_sb = cpool.tile([64, 1], F32)
            mb3_sb = cpool.tile([1, 1], F32)
            nsb_sb = cpool.tile([64, 1], F32)
            nbb_sb = cpool.tile([64, 1], F32)
            ncb_sb = cpool.tile([1, 1], F32)
            nlb_sb = cpool.tile([1, 1], F32)
            for dst_sb, src_d in ((mw1_sb, mw1), (mw2_sb, mw2), (mw3_sb, mw3),
                                  (nsw_sb, nsw), (nbw_sb, nbw), (ncw_sb, ncw),
                                  (nlw_sb, nlw), (mb1_sb, mb1), (mb2_sb, mb2),
                                  (mb3_sb, mb3), (nsb_sb, nsb), (nbb_sb, nbb),
                                  (ncb_sb, ncb), (nlb_sb, nlb)):
                nc.sync.dma_start(out=dst_sb[:], in_=src_d[:])

            with (
                tc.tile_pool(name='dps', bufs=3, space='PSUM') as psum_pool,
                tc.tile_pool(name='dstage', bufs=3) as stage_pool,
                tc.tile_pool(name='dxpage', bufs=3) as xpage_pool,
            ):
                _dense_table2(nc, tc, fTs, Wr_sb, hr_table, NK,
                              psum_pool, stage_pool, xpage_pool, brrep_sb)
                _dense_table2(nc, tc, xT, Wl_sb, hl_tabs, NROWS // P,
                              psum_pool, stage_pool, xpage_pool, blrep_sb)

            with tc.tile_pool(name='tps', bufs=2, space='PSUM') as tpsum_pool:
                def emit(k, res):
                    tp = tpsum_pool.tile([P, P], F32, tag='tp', name='tp')
                    nc.tensor.transpose(out=tp[:], in_=res[:], identity=id_sb[:])
                    nc.scalar.activation(out=fpT_sb[:, k * P:(k + 1) * P],
                                         in_=tp[:], func=AF.Copy)
                _edge_phase(nc, tc, plan, hl_tabs, hr_table, hl_idx, hr_idx,
                            dstlocT, LT, cols_sb, acc, widths, bprep_sb, emit)

            # MLP + NullModel (transposed layout; pages of 512 cols)
            with (
                tc.tile_pool(name='mps', bufs=4, space='PSUM') as mpsum,
                tc.tile_pool(name='msb', bufs=1) as msb,
            ):
                h1 = msb.tile([64, NFP], F32)
                h2 = msb.tile([64, NFP], F32)
                tot = msb.tile([1, NFP], F32)
                tmp1 = msb.tile([1, NFP], F32)
                PW = min(512, NFP)
                NPG = (NFP + PW - 1) // PW
                def _sl(pg):
                    return slice(pg * PW, min((pg + 1) * PW, NFP))
                for pg in range(NPG):
                    sl = _sl(pg)
                    w = sl.stop - sl.start
                    ps = mpsum.tile([64, PW], F32, tag='m64')
                    nc.tensor.matmul(out=ps[:, :w], lhsT=mw1_sb[:], rhs=fpT_sb[:, sl],
                                     start=True, stop=True)
                    nc.scalar.activation(out=h1[:, sl], in_=ps[:, :w], func=AF.Relu,
                                         bias=mb1_sb[:, 0:1])
                for pg in range(NPG):
                    sl = _sl(pg)
                    w = sl.stop - sl.start
                    ps = mpsum.tile([64, PW], F32, tag='m64')
                    nc.tensor.matmul(out=ps[:, :w], lhsT=mw2_sb[:], rhs=h1[:, sl],
                                     start=True, stop=True)
                    nc.scalar.activation(out=h2[:, sl], in_=ps[:, :w], func=AF.Relu,
                                         bias=mb2_sb[:, 0:1])
                for pg in range(NPG):
                    sl = _sl(pg)
                    w = sl.stop - sl.start
                    ps = mpsum.tile([1, PW], F32, tag='m1')
                    nc.tensor.matmul(out=ps[:, :w], lhsT=mw3_sb[:], rhs=h2[:, sl],
                                     start=True, stop=True)
                    nc.scalar.activation(out=tot[:, sl], in_=ps[:, :w], func=AF.Identity,
                                         bias=mb3_sb[:, 0:1])
                # NullModel
                for pg in range(NPG):
                    sl = _sl(pg)
                    w = sl.stop - sl.start
                    ps = mpsum.tile([64, PW], F32, tag='m64')
                    nc.tensor.matmul(out=ps[:, :w], lhsT=nsw_sb[:], rhs=fT_sb[:, sl],
                                     start=True, stop=True)
                    nc.scalar.activation(out=h1[:, sl], in_=ps[:, :w], func=AF.Relu,
                                         bias=nsb_sb[:, 0:1])
                for rep_i, (wsb, bsb) in enumerate(((nbw_sb, nbb_sb),
                                                    (nbw_sb, nbb_sb))):
                    src = h1 if rep_i == 0 else h2
                    dst = h2 if rep_i == 0 else h1
                    for pg in range(NPG):
                        sl = _sl(pg)
                        w = sl.stop - sl.start
                        ps = mpsum.tile([64, PW], F32, tag='m64')
                        nc.tensor.matmul(out=ps[:, :w], lhsT=wsb[:], rhs=src[:, sl],
                                         start=True, stop=True)
                        nc.scalar.activation(out=dst[:, sl], in_=ps[:, :w],
                                             func=AF.Relu, bias=bsb[:, 0:1])
                for pg in range(NPG):
                    sl = _sl(pg)
                    w = sl.stop - sl.start
                    ps = mpsum.tile([1, PW], F32, tag='m1')
                    nc.tensor.matmul(out=ps[:, :w], lhsT=ncw_sb[:], rhs=h1[:, sl],
                                     start=True, stop=True)
                    nc.scalar.activation(out=tmp1[:, sl], in_=ps[:, :w], func=AF.Identity,
                                         bias=ncb_sb[:, 0:1])
                nc.vector.tensor_tensor(out=tot[:], in0=tot[:], in1=tmp1[:],
                                        op=OP.add)
                for pg in range(NPG):
                    sl = _sl(pg)
                    w = sl.stop - sl.start
                    ps = mpsum.tile([1, PW], F32, tag='m1')
                    nc.tensor.matmul(out=ps[:, :w], lhsT=nlw_sb[:], rhs=fT_sb[:, sl],
                                     start=True, stop=True)
                    nc.scalar.activation(out=tmp1[:, sl], in_=ps[:, :w], func=AF.Identity,
                                         bias=nlb_sb[:, 0:1])
                nc.vector.tensor_tensor(out=tot[:], in0=tot[:], in1=tmp1[:],
                                        op=OP.add)
                nc.sync.dma_start(out=out[:], in_=tot[:])
    nc.compile()
    return nc


# ------------------------------------------------------------- host sim/orch
def sim_core(plan, stream, hl_tab, hr_tab, lamL, lamR, widths, bprime):
    """Numpy emulation of one core's edge phase + finalize (device-faithful)."""
    esrc, hrloc, dstloc = stream['esrc'], stream['hrloc'], stream['dstloc']
    w0, w1 = widths
    NK = plan.n_chunks
    acc = np.zeros((NK, P, 130), np.float32)
    L = (lamL[esrc] + lamR[hrloc]).astype(np.float32)
    for t, (k, m) in enumerate(plan.tiles):
        msg = hl_tab[esrc[t]].astype(np.float32)
        hrr = hr_tab[hrloc[t]].astype(np.float32)
        tt = msg + hrr
        u = np.maximum(tt, 0.0)
        RP0 = u[:, :w0].sum(1)
        RN0 = u[:, w0:64].sum(1)
        RP1 = u[:, 64:64 + w1].sum(1)
        RN1 = u[:, 64 + w1:].sum(1)
        s = np.stack([L[t, :, 0] + RP0 - RN0, L[t, :, 1] + RP1 - RN1], 1)
        e = np.exp(s).astype(np.float32)
        sel = (dstloc[t][:, None] == np.arange(P)[None, :]).astype(np.float32)
        rhs = np.concatenate([msg[:, :64] * e[:, 0:1], msg[:, 64:] * e[:, 1:2], e], 1)
        acc[k] += sel.T @ rhs
    num = acc[:, :, :128]
    den = acc[:, :, 128:130] + 1e-16
    res = np.concatenate([num[:, :, :64] / den[:, :, 0:1],
                          num[:, :, 64:] / den[:, :, 1:2]], 2)
    return (res + bprime[None, None, :]).reshape(NK * P, P).astype(np.float32)


def host_prepare(inp):
    """All host-side preprocessing independent of intermediate results."""
    f = {k: np.asarray(v) for k, v in inp.items()}
    c0 = conv_transform(f['c0_Wl'], f['c0_bl'], f['c0_Wr'], f['c0_br'],
                        f['c0_att'], f['c0_b'])
    c1 = conv_transform(f['c1_Wl'], f['c1_bl'], f['c1_Wr'], f['c1_br'],
                        f['c1_att'], f['c1_b'])
    ct = conv_transform(f['ct_Wl'], f['ct_bl'], f['ct_Wr'], f['ct_br'],
                        f['ct_att'], f['ct_b'])
    plan_pp, str_pp = build_plan_and_streams(f['epp_src'], f['epp_dst'], 98, 100352)
    plan_pf, str_pf = build_plan_and_streams(f['epf_src'], f['epf_dst'], 20, 100352)
    NPAD, FPAD = 100352, 20480
    x_pano = np.zeros((NPAD, 64), np.float32)
    x_pano[:f['x_pano'].shape[0]] = f['x_pano']
    x_fp = np.zeros((FPAD, 16), np.float32)
    x_fp[:f['x_fp'].shape[0]] = f['x_fp']
    return dict(f=f, c0=c0, c1=c1, ct=ct, plan_pp=plan_pp, str_pp=str_pp,
                plan_pf=plan_pf, str_pf=str_pf, x_pano=x_pano, x_fp=x_fp)


def layer_inputs(plan, streams, xT_full, x_slices, Wl_s, bl_s, Wr_s, br_s,
                 bprime, lamL, lamR_full):
    """Build the 8 per-core in_maps for a pano layer launch."""
    core_span = plan.n_chunks * P
    in_maps = []
    blrep_a, brrep_a = rep(bl_s), rep(br_s)
    bprep = rep(bprime)
    for c in range(N_CORES):
        st = make_stream_inputs(streams[c],
                                lamL, lamR_full[c * core_span:(c + 1) * core_span])
        in_maps.append(dict(
            xT=xT_full, xTs=x_slices[c], Wl=Wl_s, Wr=Wr_s,
            blrep=blrep_a, brrep=brrep_a,
            bprep=bprep, colsc=COLS_CONST, **st))
    return in_maps


def lam_of(x, Wlam, blam):
    return (x.astype(np.float64) @ Wlam + blam[None, :]).astype(np.float32)


def _ascontig(a):
    return np.ascontiguousarray(a, dtype=np.float32)


def run_model(inp, run_fn=None, trace=False):
    """Full 3-launch execution. run_fn(nc, in_maps) -> list of result dicts."""
    if run_fn is None:
        def run_fn(nc, in_maps):
            return bass_utils.run_bass_kernel_spmd(
                nc, in_maps, core_ids=list(range(N_CORES)), trace=trace).results
    pre = host_prepare(inp)
    f, c0, c1, ct = pre['f'], pre['c0'], pre['c1'], pre['ct']
    plan_pp, str_pp = pre['plan_pp'], pre['str_pp']
    plan_pf, str_pf = pre['plan_pf'], pre['str_pf']
    x = pre['x_pano']            # [100352, 64]
    x_fp = pre['x_fp']           # [20480, 16]
    span = 98 * P

    # ---- L1 ----
    xT = _ascontig(x.T)
    x_slices = [_ascontig(x[c * span:(c + 1) * span].T) for c in range(N_CORES)]
    lamL0 = lam_of(x, c0['Wlam_l'], c0['blam_l'])
    lamR0 = lam_of(x, c0['Wlam_r'], c0['blam_r'])
    nc1 = build_pano_layer(plan_pp, 64, c0['widths'])
    im1 = layer_inputs(plan_pp, str_pp, xT, x_slices, c0['Wl'], c0['bl'],
                       c0['Wr'], c0['br'], c0['bprime'], lamL0, lamR0)
    r1 = run_fn(nc1, im1)
    p0 = np.concatenate([r1[c]['p_out'] for c in range(N_CORES)], 0)  # [100352,128]

    # ---- L2 ----
    def rowfix(W):
        return (np.asarray(W, np.float64)[c0['perm'], :]
                / c0['A'][:, None]).astype(np.float32)
    W1l, W1r = rowfix(c1['Wl']), rowfix(c1['Wr'])
    Wlam1_l, Wlam1_r = rowfix(c1['Wlam_l']), rowfix(c1['Wlam_r'])
    lamL1 = lam_of(p0, Wlam1_l, c1['blam_l'])
    lamR1 = lam_of(p0, Wlam1_r, c1['blam_r'])
    p0T = _ascontig(p0.T)
    p0_slices = [_ascontig(r1[c]['p_out'].T) for c in range(N_CORES)]
    nc2 = build_pano_layer(plan_pp, 128, c1['widths'])
    im2 = layer_inputs(plan_pp, str_pp, p0T, p0_slices, W1l, c1['bl'],
                       W1r, c1['br'], c1['bprime'], lamL1, lamR1)
    r2 = run_fn(nc2, im2)
    p1 = np.concatenate([r2[c]['p_out'] for c in range(N_CORES)], 0)

    # ---- L3 ----
    def rowfix1(W):
        return (np.asarray(W, np.float64)[c1['perm'], :]
                / c1['A'][:, None]).astype(np.float32)
    Wtl = rowfix1(ct['Wl'])
    Wlamt_l = rowfix1(ct['Wlam_l'])
    lamLt = lam_of(p1, Wlamt_l, ct['blam_l'])
    lamRt = lam_of(x_fp, ct['Wlam_r'], ct['blam_r'])
    mw1f = input_fixup(f['m_w1'], ct['perm'], ct['A'])
    p1T = _ascontig(p1.T)
    fspan = 20 * P
    col = lambda v: _ascontig(np.asarray(v, np.float32).reshape(-1, 1))
    nc3 = build_l3(plan_pf, 128, ct['widths'])
    im3 = []
    for c in range(N_CORES):
        st = make_stream_inputs(str_pf[c], lamLt,
                                lamRt[c * fspan:(c + 1) * fspan])
        im3.append(dict(
            xT=p1T, fTs=_ascontig(x_fp[c * fspan:(c + 1) * fspan].T),
            Wl=Wtl, Wr=ct['Wr'],
            blrep=rep(ct['bl']), brrep=rep(ct['br']),
            bprep=rep(ct['bprime']), colsc=COLS_CONST,
            ident=np.eye(P, dtype=np.float32),
            mw1=mw1f, mb1=col(f['m_b1']), mw2=_ascontig(f['m_w2']),
            mb2=col(f['m_b2']), mw3=_ascontig(f['m_w3']), mb3=col(f['m_b3']),
            nsw=_ascontig(f['nm_sw']), nsb=col(f['nm_sb']),
            nbw=_ascontig(f['nm_bw']), nbb=col(f['nm_bb']),
            ncw=_ascontig(f['nm_cw']), ncb=col(f['nm_cb']),
            nlw=_ascontig(f['nm_lw']), nlb=col(f['nm_lb']), **st))
    r3 = run_fn(nc3, im3)
    out = np.concatenate([r3[c]['out'][0] for c in range(N_CORES)])
    return out[:20000].reshape(20000, 1).astype(np.float32)


# ---------------------------------------------------------------- kernel API
def kernel(**inputs):
    """Self-contained entry: full inputs -> full [20000, 1] float32 output."""
    return run_model(inputs)



# revision 35
# speedup vs baseline: 1.9295x; 1.9295x over previous
"""CustomGAT on 8 trn2 cores — v2 (gather-transpose + on-the-fly projection).

Three SPMD launches:
  L1: pano GAT layer 0   (table: x_pano bf16-padded)  -> p0 bf16
  L2: pano GAT layer 1   (same compiled program, table: p0)
  L3: translate conv + NullModel + closing MLP        -> [1, 2560] f32 slices

Per-layer device program (all bf16 compute, f32 psum):
  dense-lite: hr'[n] = x_loc[n]@Wr + (bl+br) into SBUF; hrb = b' - hr'.
  edge phase, dst-partitioned, (chunk,window)-pure 128-edge tiles:
    xgT   <- dma_gather(transpose=True) from the node table (4 windows)
    t_ps   = xgT^T@Wl + selT^T@hr'[chunk]      (PE, psum accum)
    u      = relu(t_ps)                        (Act)
    s      = lam + signed col-group reduces(u) (DVE)
    e2     = exp(s)                            (Act)
    rhs    = [t_ps*e2 | e2]                    (DVE)
    run[k]+= sel^T@rhs                         (PE, psum run per chunk)
  finalize: res = run*rec(den) + hrb[k]  (alpha sums to 1 => -hr' correction);
  degree-0 dsts get injected zero-row edges so the identity holds.
"""
import numpy as np
import ml_dtypes

import concourse.bass as bass
import concourse.bacc as bacc
import concourse.mybir as mybir
from concourse.tile import TileContext
from concourse.vector_clock import ScopedClock
from concourse import bass_utils

F32 = mybir.dt.float32
BF16 = mybir.dt.bfloat16
I16 = mybir.dt.int16
AF = mybir.ActivationFunctionType
OP = mybir.AluOpType
NPBF = ml_dtypes.bfloat16

P = 128
N_CORES = 8
NROWS = 100352          # padded pano rows (also L3 table rows)
WIN = 25088             # gather window rows (196*128, < 32768)
NWIN = 4
B_GRP = 6               # chunks per psum group (2 run tiles x 3 slots)
G = 24                  # max tiles per gather batch
SUB = 8                 # tiles per compute subgroup
ZROW = NROWS - 1        # guaranteed-zero table row for injected edges


# ---------------------------------------------------------------- drain patch
def _patched_drain_and_barrier(self, tick_clock, wait_clock):
    victim = self.nc.sync.nop(nofuse=True)
    wait_clock.add_sem_waits(victim.ins, ScopedClock({None: tick_clock.global_clock}))
    si = victim.ins.sync_info
    waits = list(si.on_wait) if si is not None and si.on_wait else []
    if si is not None and len(waits) > 1:
        si.on_wait = waits[:1]
        for w in waits[1:]:
            extra = self.nc.sync.nop(nofuse=True)
            esi = extra.ins.sync_info
            if esi is None:
                extra.ins.sync_info = mybir.SyncInfo(on_wait=[w], on_update=[])
            else:
                esi.on_wait = [w]
    self.nc.sync.drain()
    self.nc.all_engine_barrier()
    popped = self.nc._tile_sem_poison_stack.pop()
    assert popped is self._sem_poison
    self.nc.clear_and_free_semaphores(list(self.sems.allocated().values()))
    self.nc.all_engine_barrier()


TileContext._drain_and_barrier = _patched_drain_and_barrier


# ---------------------------------------------------------------- host: plan
class Plan:
    __slots__ = ('NK', 'T', 'attrs', 'groups', 'batches', 'gt_max')

    def __init__(self, **kw):
        for k, v in kw.items():
            setattr(self, k, v)


def build_plan(src, dst, n_chunks):
    """(chunk,window)-pure tile plan, group-window-major stream order.

    Returns (plan, per-core streams). Structure (tile counts/order) is shared
    across cores (max over cores per (k,w) cell); streams are per-core.
    """
    src = np.asarray(src, np.int64)
    dst = np.asarray(dst, np.int64)
    span = n_chunks * P
    order = np.argsort(dst, kind='stable')
    s_src, s_dst = src[order], dst[order]

    counts = np.zeros((N_CORES, n_chunks, NWIN), np.int64)
    per_core = []
    for c in range(N_CORES):
        lo = np.searchsorted(s_dst, c * span, 'left')
        hi = np.searchsorted(s_dst, (c + 1) * span, 'left')
        cs, cd = s_src[lo:hi], s_dst[lo:hi]
        k = (cd - c * span) >> 7
        w = cs // WIN
        o2 = np.argsort(k * NWIN + w, kind='stable')
        cs, cd = cs[o2], cd[o2]
        key = (k * NWIN + w)[o2]
        bounds = np.searchsorted(key, np.arange(n_chunks * NWIN + 1))
        counts[c] = np.diff(bounds).reshape(n_chunks, NWIN)
        per_core.append((cs, cd, bounds))
    t_kw = -(-counts.max(0) // P)          # [NK, NWIN]

    tiles = []        # (k, w)
    tstart = {}       # (k, w) -> first tile index
    groups = []       # (t0, t1)
    batches = []      # (w, t0, nt)
    for g0 in range(0, n_chunks, B_GRP):
        ks = range(g0, min(g0 + B_GRP, n_chunks))
        g_t0 = len(tiles)
        for w in range(NWIN):
            bt0 = len(tiles)
            for k in ks:
                if t_kw[k, w]:
                    tstart[(k, w)] = len(tiles)
                    tiles.extend([(k, w)] * int(t_kw[k, w]))
            t = bt0
            while t < len(tiles):
                nt = min(G, len(tiles) - t)
                batches.append((w, t, nt))
                t += nt
        groups.append((g_t0, len(tiles)))
    T = len(tiles)
    run_first, run_last = {}, {}
    for t, (k, _) in enumerate(tiles):
        run_first.setdefault(k, t)
        run_last[k] = t
    attrs = []
    for t, (k, _w) in enumerate(tiles):
        slot = k - (k // B_GRP) * B_GRP
        attrs.append((k, _w, slot, t == run_first[k], t == run_last[k]))
    gt_max = max(t1 - t0 for t0, t1 in groups)
    plan = Plan(NK=n_chunks, T=T, attrs=attrs, groups=groups, batches=batches,
                gt_max=gt_max)

    streams = []
    for c in range(N_CORES):
        cs, cd, bounds = per_core[c]
        idxl = np.zeros((T, P), np.int64)
        srcg = np.zeros((T, P), np.int64)
        dloc = np.full((T, P), -1.0, np.float32)
        dglo = np.zeros((T, P), np.int64)
        pad = np.ones((T, P), bool)
        for k in range(n_chunks):
            for w in range(NWIN):
                tk = int(t_kw[k, w])
                if tk == 0:
                    continue
                i0, i1 = bounds[k * NWIN + w], bounds[k * NWIN + w + 1]
                es, ed = cs[i0:i1], cd[i0:i1]
                n = i1 - i0
                t0 = tstart[(k, w)]
                bi = np.zeros(tk * P, np.int64)
                bi[:n] = es - w * WIN
                bs = np.full(tk * P, w * WIN, np.int64)
                bs[:n] = es
                bl = np.full(tk * P, -1.0, np.float32)
                bl[:n] = ed - (c * span + k * P)
                bg = np.zeros(tk * P, np.int64)
                bg[:n] = ed
                bp = np.ones(tk * P, bool)
                bp[:n] = False
                idxl[t0:t0 + tk] = bi.reshape(tk, P)
                srcg[t0:t0 + tk] = bs.reshape(tk, P)
                dloc[t0:t0 + tk] = bl.reshape(tk, P)
                dglo[t0:t0 + tk] = bg.reshape(tk, P)
                pad[t0:t0 + tk] = bp.reshape(tk, P)
        streams.append(dict(idxl=idxl, srcg=srcg, dloc=dloc, dglo=dglo, pad=pad))
    return plan, streams


def wrap_idx16(flat_idx):
    """[T,128] -> [128, T*8] int16 dma_gather layout (16-wrap, x8 replicated)."""
    n = flat_idx.size
    x = flat_idx.reshape(n)
    w = np.zeros((16, n // 16), np.int16)
    pos = np.arange(n)
    w[pos % 16, pos // 16] = x.astype(np.int16)
    return np.tile(w, (8, 1))


def make_sel_streams(stream):
    """sel [128e, T*128d] and selT [128d, T*128e] one-hot streams (bf16)."""
    dloc = stream['dloc'].astype(np.int64)            # [T, 128], -1 pads
    T = dloc.shape[0]
    eye = np.arange(P, dtype=np.int64)
    sel3 = (dloc[:, :, None] == eye[None, None, :])   # [T, e, d]
    sel = np.ascontiguousarray(
        sel3.transpose(1, 0, 2).reshape(P, T * P).astype(NPBF))
    selT = np.ascontiguousarray(
        sel3.transpose(2, 0, 1).reshape(P, T * P).astype(NPBF))
    return sel, selT


def make_stream_inputs(stream, lamL, lamR_glob):
    """Per-core per-layer stream arrays: idx, sel/selT, LT."""
    T = stream['idxl'].shape[0]
    hl_idx = wrap_idx16(stream['idxl'])
    if 'sel' not in stream:
        stream['sel'], stream['selT'] = make_sel_streams(stream)
    L = (lamL[stream['srcg']] + lamR_glob[stream['dglo']]).astype(np.float32)
    L[stream['pad']] = -30000.0
    LT = np.ascontiguousarray(L.transpose(1, 0, 2).reshape(P, T * 2))
    return dict(hl_idx=hl_idx, sel_s=stream['sel'], selT_s=stream['selT'],
                LT=LT)


# ---------------------------------------------------------- conv transforms
def conv_transform(Wl, bl, Wr, br, att, b):
    H, C = att.shape
    a = np.asarray(att, np.float64).reshape(-1)
    perm, widths = [], []
    for h in range(H):
        cols = np.arange(h * C, (h + 1) * C)
        pos = cols[a[cols] >= 0]
        neg = cols[a[cols] < 0]
        widths.append(len(pos))
        perm.extend(pos.tolist())
        perm.extend(neg.tolist())
    perm = np.array(perm, np.int64)
    A = np.maximum(0.8 * np.abs(a[perm]), 1e-12)

    def scale_cols(W, bvec):
        W = np.asarray(W, np.float64)
        bvec = np.asarray(bvec, np.float64)
        return ((W[:, perm] * A[None, :]).astype(np.float32),
                (bvec[perm] * A).astype(np.float32))

    Wl_s, bl_s = scale_cols(Wl, bl)
    Wr_s, br_s = scale_cols(Wr, br)
    Wlam_l = np.stack([0.2 * (np.asarray(Wl, np.float64)[:, h * C:(h + 1) * C]
                              @ a[h * C:(h + 1) * C]) for h in range(H)], 1)
    blam_l = np.array([0.2 * (np.asarray(bl, np.float64)[h * C:(h + 1) * C]
                              @ a[h * C:(h + 1) * C]) for h in range(H)])
    Wlam_r = np.stack([0.2 * (np.asarray(Wr, np.float64)[:, h * C:(h + 1) * C]
                              @ a[h * C:(h + 1) * C]) for h in range(H)], 1)
    blam_r = np.array([0.2 * (np.asarray(br, np.float64)[h * C:(h + 1) * C]
                              @ a[h * C:(h + 1) * C]) for h in range(H)])
    bprime = (np.asarray(b, np.float64)[perm] * A).astype(np.float32)
    return dict(perm=perm, A=A, widths=widths, Wl=Wl_s, bl=bl_s, Wr=Wr_s,
                br=br_s, Wlam_l=Wlam_l, blam_l=blam_l, Wlam_r=Wlam_r,
                blam_r=blam_r, bprime=bprime)


def input_fixup(W, perm, A):
    W = np.asarray(W, np.float64)
    return (W[perm, :] / A[:, None]).astype(np.float32)


def lam_of(x, Wlam, blam):
    return (np.asarray(x, np.float64) @ Wlam + blam[None, :]).astype(np.float32)


def to_bf(a):
    return np.ascontiguousarray(np.asarray(a, np.float32).astype(NPBF))


def pad128(W):
    """[d, 128] -> [128, 128] with zero rows below d."""
    W = np.asarray(W, np.float32)
    out = np.zeros((P, P), np.float32)
    out[:W.shape[0]] = W
    return out


COLS_CONST = np.repeat(np.arange(P, dtype=np.float32)[None, :], P, 0)


# ------------------------------------------------------------ device builder
def _dense_hr(nc, tc, xTs, Wr_sb, ones1_sb, brow_sb, bprep_sb, hrp_sb, hrb_sb,
              NK):
    """hr'[n] = x_loc[n]@Wr + (bl+br); hrb = b' - hr' (both SBUF resident)."""
    with (
        tc.tile_pool(name='dxp', bufs=2) as xpool,
        tc.tile_pool(name='dps', bufs=2, space='PSUM') as pspool,
    ):
        PG = 8
        for pg0 in range(0, NK, PG):
            jn = min(PG, NK - pg0)
            xp = xpool.tile([P, PG * P], BF16, tag='xp')
            nc.sync.dma_start(out=xp[:, :jn * P],
                              in_=xTs[:, pg0 * P:(pg0 + jn) * P])
            for s0 in range(0, jn, SUB):
                ns = min(SUB, jn - s0)
                ps = pspool.tile([P, SUB * P], F32, tag='dhr', name='dhr')
                for j in range(ns):
                    nc.tensor.matmul(out=ps[:, j * P:(j + 1) * P],
                                     lhsT=xp[:, (s0 + j) * P:(s0 + j + 1) * P],
                                     rhs=Wr_sb[:], start=True, stop=False)
                    nc.tensor.matmul(out=ps[:, j * P:(j + 1) * P],
                                     lhsT=ones1_sb[:], rhs=brow_sb[:],
                                     start=False, stop=True)
                k0 = pg0 + s0
                nc.scalar.activation(out=hrp_sb[:, k0 * P:(k0 + ns) * P],
                                     in_=ps[:, :ns * P], func=AF.Copy)
                nc.vector.tensor_tensor(
                    out=hrb_sb[:, k0 * P:(k0 + ns) * P].rearrange(
                        "p (j c) -> p j c", c=P),
                    in0=bprep_sb[:].rearrange("p (o c) -> p o c", o=1)
                    .to_broadcast([P, ns, P]),
                    in1=hrp_sb[:, k0 * P:(k0 + ns) * P].rearrange(
                        "p (j c) -> p j c", c=P),
                    op=OP.subtract)


def _edge_phase(nc, tc, plan, tab, hl_idx, sel_str, selT_str, LT, ident_sb,
                Wl_sb, hrp_sb, hrb_sb, ones1_sb, z130_sb, widths,
                alloc_cb, emit_cb):
    w0, w1 = widths
    ranges = [(0, w0), (64, 64 + w1), (w0, 64), (64 + w1, P)]  # RP0 RP1 RN0 RN1
    GT = plan.gt_max
    with (
        tc.tile_pool(name='est', bufs=2) as stream_pool,
        tc.tile_pool(name='exg', bufs=6) as xg_pool,
        tc.tile_pool(name='esel', bufs=4) as sel_pool,
        tc.tile_pool(name='etps', bufs=3, space='PSUM') as t_psum,
        tc.tile_pool(name='eu', bufs=4) as u_pool,
        tc.tile_pool(name='ered', bufs=4) as red_pool,
        tc.tile_pool(name='erhs', bufs=4) as rhs_pool,
        tc.tile_pool(name='erun', bufs=2, space='PSUM') as run_pool,
        tc.tile_pool(name='efin', bufs=4) as fin_pool,
    ):
        n_batches = len(plan.batches)
        bi = 0
        for gi, (g_t0, g_t1) in enumerate(plan.groups):
            gt = g_t1 - g_t0
            ltt_sb = stream_pool.tile([P, GT * 2], F32, tag='ltt')
            nc.sync.dma_start(out=ltt_sb[:, :gt * 2],
                              in_=LT[:, g_t0 * 2:g_t1 * 2])
            idx_sb = stream_pool.tile([P, GT * 8], I16, tag='idx')
            nc.sync.dma_start(out=idx_sb[:, :gt * 8],
                              in_=hl_idx[:, g_t0 * 8:g_t1 * 8])
            rt = [run_pool.tile([P, 3 * 130], F32, tag='rt', name='rt')
                  for _ in range(2)]
            # zero all run regions up-front: interleaved psum accumulation
            # groups must not issue start=True into a bank holding live
            # partials of sibling regions (start zeroes the whole bank).
            nused = min(B_GRP, plan.NK - gi * B_GRP)
            for sl in range(2 * 3):
                nc.tensor.matmul(
                    out=rt[sl // 3][:, (sl % 3) * 130:(sl % 3) * 130 + 130],
                    lhsT=ones1_sb[:], rhs=z130_sb[:],
                    start=True, stop=sl >= nused)
            while bi < n_batches and plan.batches[bi][1] < g_t1:
                w, t0, nt = plan.batches[bi]
                bi += 1
                bo = t0 - g_t0
                xg = xg_pool.tile([P, G * P], BF16, tag='xg')
                nc.gpsimd.dma_gather(
                    out_ap=xg[:, :nt * P].rearrange("p (o n) -> p o n", o=1),
                    in_ap=tab[w * WIN:(w + 1) * WIN, :],
                    idxs_ap=idx_sb[:, bo * 8:(bo + nt) * 8],
                    num_idxs=nt * P, num_idxs_reg=nt * P,
                    elem_size=P, transpose=True, single_packet=False)
                sel_sb = sel_pool.tile([P, G * P], BF16, tag='sel')
                sts_sb = sel_pool.tile([P, G * P], BF16, tag='sts')
                nc.scalar.dma_start(out=sel_sb[:, :nt * P],
                                    in_=sel_str[:, t0 * P:(t0 + nt) * P])
                nc.scalar.dma_start(out=sts_sb[:, :nt * P],
                                    in_=selT_str[:, t0 * P:(t0 + nt) * P])
                for s0 in range(0, nt, SUB):
                    ns = min(SUB, nt - s0)
                    q0 = t0 + s0
                    go = q0 - g_t0
                    tps = t_psum.tile([P, SUB * P], F32, tag='tps', name='tps')
                    for j in range(ns):
                        k = plan.attrs[q0 + j][0]
                        nc.tensor.matmul(
                            out=tps[:, j * P:(j + 1) * P],
                            lhsT=xg[:, (s0 + j) * P:(s0 + j + 1) * P],
                            rhs=Wl_sb[:], start=True, stop=False)
                        nc.tensor.matmul(
                            out=tps[:, j * P:(j + 1) * P],
                            lhsT=sts_sb[:, (s0 + j) * P:(s0 + j + 1) * P],
                            rhs=hrp_sb[:, k * P:(k + 1) * P],
                            start=False, stop=True)
                    u = u_pool.tile([P, SUB * P], BF16, tag='u')
                    tb = u_pool.tile([P, SUB * P], BF16, tag='tb')
                    nc.scalar.activation(out=u[:, :ns * P], in_=tps[:, :ns * P],
                                         func=AF.Relu)
                    nc.scalar.activation(out=tb[:, :ns * P],
                                         in_=tps[:, :ns * P], func=AF.Copy)
                    red = red_pool.tile([P, SUB * 4], F32, tag='red')
                    s_sb = red_pool.tile([P, SUB * 2], F32, tag='s')
                    uv = u[:, :ns * P].rearrange("p (j c) -> p j c", c=P)
                    rv = red[:, :ns * 4].rearrange("p (j f) -> p j f", f=4)
                    for ri, (c0, c1) in enumerate(ranges):
                        nc.vector.tensor_reduce(
                            out=rv[:, :, ri:ri + 1], in_=uv[:, :, c0:c1],
                            axis=mybir.AxisListType.X, op=OP.add)
                    sv = s_sb[:, :ns * 2].rearrange("p (j h) -> p j h", h=2)
                    lv = ltt_sb[:, go * 2:(go + ns) * 2].rearrange(
                        "p (j h) -> p j h", h=2)
                    nc.vector.tensor_tensor(out=sv, in0=lv, in1=rv[:, :, 0:2],
                                            op=OP.add)
                    nc.vector.tensor_tensor(out=sv, in0=sv, in1=rv[:, :, 2:4],
                                            op=OP.subtract)
                    rhs = rhs_pool.tile([P, SUB * 130], BF16, tag='rhs')
                    rview = rhs[:, :ns * 130].rearrange("p (j c) -> p j c",
                                                        c=130)
                    nc.scalar.activation(out=rview[:, :, 128:130], in_=sv,
                                         func=AF.Exp)
                    nc.vector.tensor_tensor(
                        out=rview[:, :, 0:128].rearrange(
                            "p j (h c) -> p j h c", c=64),
                        in0=tb[:, :ns * P].rearrange(
                            "p (j h c) -> p j h c", h=2, c=64),
                        in1=rview[:, :, 128:130].rearrange(
                            "p j (h o) -> p j h o", o=1)
                        .to_broadcast([P, ns, 2, 64]),
                        op=OP.mult)
                    for j in range(ns):
                        k, _w, slot, rs, re = plan.attrs[q0 + j]
                        run = rt[slot // 3]
                        off = (slot % 3) * 130
                        nc.tensor.matmul(out=run[:, off:off + 130],
                                         lhsT=sel_sb[:, (s0 + j) * P:
                                                     (s0 + j + 1) * P],
                                         rhs=rhs[:, j * 130:(j + 1) * 130],
                                         start=False, stop=re)
                        if re:
                            _finalize(nc, fin_pool, run, off, hrb_sb, k,
                                      alloc_cb, emit_cb)


def _finalize(nc, fin_pool, run, off, hrb_sb, k, alloc_cb, emit_cb):
    dadj = fin_pool.tile([P, 2], F32, tag='dadj', name='dadj')
    rec = fin_pool.tile([P, 2], F32, tag='rec', name='rec')
    nc.vector.tensor_scalar_add(out=dadj[:], in0=run[:, off + 128:off + 130],
                                scalar1=1e-16)
    nc.vector.reciprocal(out=rec[:], in_=dadj[:])
    res = alloc_cb(k)
    for h in (0, 1):
        nc.vector.scalar_tensor_tensor(
            out=res[:, h * 64:(h + 1) * 64],
            in0=run[:, off + h * 64:off + h * 64 + 64],
            scalar=rec[:, h:h + 1],
            in1=hrb_sb[:, k * P + h * 64:k * P + h * 64 + 64],
            op0=OP.mult, op1=OP.add)
    emit_cb(k, res)


def build_gat(plan, widths, l3=False):
    nc = bacc.Bacc("TRN2", target_bir_lowering=False, debug=False,
                   num_devices=N_CORES)
    NK = plan.NK
    T = plan.T
    NFP = NK * P
    tab = nc.dram_tensor('tab', [NROWS, P], BF16, kind='ExternalInput')
    xTs = nc.dram_tensor('xTs', [P, NK * P], BF16, kind='ExternalInput')
    Wl = nc.dram_tensor('Wl', [P, P], BF16, kind='ExternalInput')
    Wr = nc.dram_tensor('Wr', [P, P], BF16, kind='ExternalInput')
    brow = nc.dram_tensor('brow', [1, P], BF16, kind='ExternalInput')
    ones1 = nc.dram_tensor('ones1', [1, P], BF16, kind='ExternalInput')
    z130 = nc.dram_tensor('z130', [1, 130], BF16, kind='ExternalInput')
    bprep = nc.dram_tensor('bprep', [P, P], BF16, kind='ExternalInput')
    ident = nc.dram_tensor('ident', [P, P], BF16, kind='ExternalInput')
    hl_idx = nc.dram_tensor('hl_idx', [P, T * 8], I16, kind='ExternalInput')
    sel_str = nc.dram_tensor('sel_s', [P, T * P], BF16, kind='ExternalInput')
    selT_str = nc.dram_tensor('selT_s', [P, T * P], BF16,
                              kind='ExternalInput')
    LT = nc.dram_tensor('LT', [P, T * 2], F32, kind='ExternalInput')
    if not l3:
        p_out = nc.dram_tensor('p_out', [NK * P, P], BF16,
                               kind='ExternalOutput')
    else:
        fT = nc.dram_tensor('fT', [16, NFP], BF16, kind='ExternalInput')
        mw1 = nc.dram_tensor('mw1', [P, 64], BF16, kind='ExternalInput')
        mw2 = nc.dram_tensor('mw2', [64, 64], BF16, kind='ExternalInput')
        mw3 = nc.dram_tensor('mw3', [64, 1], BF16, kind='ExternalInput')
        nsw = nc.dram_tensor('nsw', [16, 64], BF16, kind='ExternalInput')
        nbw = nc.dram_tensor('nbw', [64, 64], BF16, kind='ExternalInput')
        ncw = nc.dram_tensor('ncw', [64, 1], BF16, kind='ExternalInput')
        nlw = nc.dram_tensor('nlw', [16, 1], BF16, kind='ExternalInput')
        mb1 = nc.dram_tensor('mb1', [64, 1], F32, kind='ExternalInput')
        mb2 = nc.dram_tensor('mb2', [64, 1], F32, kind='ExternalInput')
        mb3 = nc.dram_tensor('mb3', [1, 1], F32, kind='ExternalInput')
        nsb = nc.dram_tensor('nsb', [64, 1], F32, kind='ExternalInput')
        nbb = nc.dram_tensor('nbb', [64, 1], F32, kind='ExternalInput')
        ncb = nc.dram_tensor('ncb', [1, 1], F32, kind='ExternalInput')
        nlb = nc.dram_tensor('nlb', [1, 1], F32, kind='ExternalInput')
        out = nc.dram_tensor('out', [1, NFP], F32, kind='ExternalOutput')

    with TileContext(nc) as tc:
        with tc.tile_pool(name='const', bufs=1) as cpool:
            Wl_sb = cpool.tile([P, P], BF16)
            Wr_sb = cpool.tile([P, P], BF16)
            brow_sb = cpool.tile([1, P], BF16)
            ones1_sb = cpool.tile([1, P], BF16)
            bprep_sb = cpool.tile([P, P], BF16)
            ident_sb = cpool.tile([P, P], BF16)
            hrp_sb = cpool.tile([P, NK * P], BF16)
            hrb_sb = cpool.tile([P, NK * P], BF16)
            z130_sb = cpool.tile([1, 130], BF16)
            loads = [(Wl_sb, Wl), (Wr_sb, Wr), (brow_sb, brow),
                     (ones1_sb, ones1), (bprep_sb, bprep),
                     (ident_sb, ident), (z130_sb, z130)]
            if l3:
                fT_sb = cpool.tile([16, NFP], BF16)
                fp_sb = cpool.tile([P, NK * P], BF16)
                fpT_sb = cpool.tile([P, NK * P], BF16)
                mw1_sb = cpool.tile([P, 64], BF16)
                mw2_sb = cpool.tile([64, 64], BF16)
                mw3_sb = cpool.tile([64, 1], BF16)
                nsw_sb = cpool.tile([16, 64], BF16)
                nbw_sb = cpool.tile([64, 64], BF16)
                ncw_sb = cpool.tile([64, 1], BF16)
                nlw_sb = cpool.tile([16, 1], BF16)
                mb1_sb = cpool.tile([64, 1], F32)
                mb2_sb = cpool.tile([64, 1], F32)
                mb3_sb = cpool.tile([1, 1], F32)
                nsb_sb = cpool.tile([64, 1], F32)
                nbb_sb = cpool.tile([64, 1], F32)
                ncb_sb = cpool.tile([1, 1], F32)
                nlb_sb = cpool.tile([1, 1], F32)
                loads += [(fT_sb, fT), (mw1_sb, mw1), (mw2_sb, mw2),
                          (mw3_sb, mw3), (nsw_sb, nsw), (nbw_sb, nbw),
                          (ncw_sb, ncw), (nlw_sb, nlw), (mb1_sb, mb1),
                          (mb2_sb, mb2), (mb3_sb, mb3), (nsb_sb, nsb),
                          (nbb_sb, nbb), (ncb_sb, ncb), (nlb_sb, nlb)]
            for dst_sb, src_d in loads:
                nc.sync.dma_start(out=dst_sb[:], in_=src_d[:])

            _dense_hr(nc, tc, xTs, Wr_sb, ones1_sb, brow_sb, bprep_sb,
                      hrp_sb, hrb_sb, NK)

            if not l3:
                with tc.tile_pool(name='eres', bufs=3) as res_pool:
                    def alloc_cb(k):
                        return res_pool.tile([P, P], BF16, tag='res',
                                             name='res')

                    def emit_cb(k, res):
                        nc.sync.dma_start(out=p_out[k * P:(k + 1) * P, :],
                                          in_=res[:])
                    _edge_phase(nc, tc, plan, tab, hl_idx, sel_str, selT_str,
                                LT, ident_sb, Wl_sb, hrp_sb, hrb_sb,
                                ones1_sb, z130_sb, widths, alloc_cb, emit_cb)
            else:
                def alloc_cb(k):
                    return fp_sb[:, k * P:(k + 1) * P]

                def emit_cb(k, res):
                    pass
                _edge_phase(nc, tc, plan, tab, hl_idx, sel_str, selT_str,
                            LT, ident_sb, Wl_sb, hrp_sb, hrb_sb,
                            ones1_sb, z130_sb, widths, alloc_cb, emit_cb)
                # transpose fp -> fpT for the MLP
                with (
                    tc.tile_pool(name='tps2', bufs=2, space='PSUM') as tpool2,
                ):
                    for k in range(NK):
                        tp = tpool2.tile([P, P], BF16, tag='tp', name='tp')
                        nc.tensor.transpose(out=tp[:],
                                            in_=fp_sb[:, k * P:(k + 1) * P],
                                            identity=ident_sb[:])
                        nc.scalar.activation(out=fpT_sb[:, k * P:(k + 1) * P],
                                             in_=tp[:], func=AF.Copy)
                # MLP + NullModel (transposed layout; pages of 512 cols)
                with (
                    tc.tile_pool(name='mps', bufs=4, space='PSUM') as mpsum,
                    tc.tile_pool(name='msb', bufs=1) as msb,
                ):
                    h1 = msb.tile([64, NFP], BF16)
                    h2 = msb.tile([64, NFP], BF16)
                    tot = msb.tile([1, NFP], F32)
                    tmp1 = msb.tile([1, NFP], F32)
                    PW = min(512, NFP)
                    NPG = (NFP + PW - 1) // PW

                    def _mlp_pass(w_sb, b_sb, src, dst, func, width=64):
                        for pg in range(NPG):
                            sl = slice(pg * PW, min((pg + 1) * PW, NFP))
                            wd = sl.stop - sl.start
                            ps = mpsum.tile([width, PW], F32,
                                            tag=f'm{width}', name='ps')
                            nc.tensor.matmul(out=ps[:, :wd], lhsT=w_sb[:],
                                             rhs=src[:, sl], start=True,
                                             stop=True)
                            nc.scalar.activation(out=dst[:, sl],
                                                 in_=ps[:, :wd], func=func,
                                                 bias=b_sb[:, 0:1])
                    _mlp_pass(mw1_sb, mb1_sb, fpT_sb, h1, AF.Relu)
                    _mlp_pass(mw2_sb, mb2_sb, h1, h2, AF.Relu)
                    _mlp_pass(mw3_sb, mb3_sb, h2, tot, AF.Identity, width=1)
                    _mlp_pass(nsw_sb, nsb_sb, fT_sb, h1, AF.Relu)
                    _mlp_pass(nbw_sb, nbb_sb, h1, h2, AF.Relu)
                    _mlp_pass(nbw_sb, nbb_sb, h2, h1, AF.Relu)
                    _mlp_pass(ncw_sb, ncb_sb, h1, tmp1, AF.Identity, width=1)
                    nc.vector.tensor_tensor(out=tot[:], in0=tot[:],
                                            in1=tmp1[:], op=OP.add)
                    _mlp_pass(nlw_sb, nlb_sb, fT_sb, tmp1, AF.Identity,
                              width=1)
                    nc.vector.tensor_tensor(out=tot[:], in0=tot[:],
                                            in1=tmp1[:], op=OP.add)
                    nc.sync.dma_start(out=out[:], in_=tot[:])
    nc.compile()
    return nc


# ------------------------------------------------------------- host orch
def host_prepare(inp):
    f = {k: np.asarray(v) for k, v in inp.items()}
    c0 = conv_transform(f['c0_Wl'], f['c0_bl'], f['c0_Wr'], f['c0_br'],
                        f['c0_att'], f['c0_b'])
    c1 = conv_transform(f['c1_Wl'], f['c1_bl'], f['c1_Wr'], f['c1_br'],
                        f['c1_att'], f['c1_b'])
    ct = conv_transform(f['ct_Wl'], f['ct_bl'], f['ct_Wr'], f['ct_br'],
                        f['ct_att'], f['ct_b'])
    N, NFP = 100000, 20000

    def inject(src, dst, ndst):
        deg = np.bincount(dst, minlength=ndst)
        empty = np.nonzero(deg == 0)[0]
        if len(empty):
            src = np.concatenate([src, np.full(len(empty), ZROW)])
            dst = np.concatenate([dst, empty])
        return src, dst

    pp_src, pp_dst = inject(f['epp_src'].astype(np.int64),
                            f['epp_dst'].astype(np.int64), N)
    pf_src, pf_dst = inject(f['epf_src'].astype(np.int64),
                            f['epf_dst'].astype(np.int64), NFP)
    plan_pp, str_pp = build_plan(pp_src, pp_dst, 98)
    plan_pf, str_pf = build_plan(pf_src, pf_dst, 20)

    x_pad = np.zeros((NROWS, P), np.float32)
    x_pad[:N, :64] = f['x_pano']
    x_pad_bf = to_bf(x_pad)
    x_fp_pad = np.zeros((20480, 16), np.float32)
    x_fp_pad[:NFP] = f['x_fp']
    x_fp_bf = to_bf(x_fp_pad)
    return dict(f=f, c0=c0, c1=c1, ct=ct, plan_pp=plan_pp, str_pp=str_pp,
                plan_pf=plan_pf, str_pf=str_pf, x_pad_bf=x_pad_bf,
                x_fp_bf=x_fp_bf)


def layer_in_maps(plan, streams, tab_bf, Wl_s, Wr_s, bl_s, br_s, bprime,
                  lamL, lamR, extras=None):
    span = plan.NK * P
    Wl_a = to_bf(pad128(Wl_s))
    Wr_a = to_bf(pad128(Wr_s))
    brow = to_bf((np.asarray(bl_s) + np.asarray(br_s)).reshape(1, P))
    ones1 = to_bf(np.ones((1, P), np.float32))
    z130 = to_bf(np.zeros((1, 130), np.float32))
    bprep = to_bf(np.repeat(np.asarray(bprime, np.float32).reshape(1, P),
                            P, 0))
    ident = to_bf(np.eye(P, dtype=np.float32))
    in_maps = []
    for c in range(N_CORES):
        st = make_stream_inputs(streams[c], lamL, lamR)
        m = dict(tab=tab_bf, Wl=Wl_a, Wr=Wr_a, brow=brow, ones1=ones1,
                 z130=z130, bprep=bprep, ident=ident, **st)
        if extras is not None:
            m.update(extras[c])
        in_maps.append(m)
    return in_maps


def _xts_slice(tab_f32, c, span):
    """[span, <=128] -> [128, span] bf16 (zero-pad rows)."""
    sl = np.zeros((P, span), np.float32)
    blk = tab_f32[c * span:(c + 1) * span]
    sl[:blk.shape[1], :] = blk.T
    return to_bf(sl)


_NC_CACHE = {}


def run_model(inp, run_fn=None, trace=False):
    if run_fn is None:
        def run_fn(nc, in_maps):
            return bass_utils.run_bass_kernel_spmd(
                nc, in_maps, core_ids=list(range(N_CORES)), trace=trace).results
    pre = host_prepare(inp)
    f, c0, c1, ct = pre['f'], pre['c0'], pre['c1'], pre['ct']
    plan_pp, str_pp = pre['plan_pp'], pre['str_pp']
    plan_pf, str_pf = pre['plan_pf'], pre['str_pf']
    span = 98 * P
    fspan = 20 * P
    key1 = ('pano', plan_pp.T, tuple(c0['widths']))
    if key1 not in _NC_CACHE:
        _NC_CACHE[key1] = build_gat(plan_pp, tuple(c0['widths']), l3=False)
    nc1 = _NC_CACHE[key1]
    key2 = ('pano', plan_pp.T, tuple(c1['widths']))
    if key2 not in _NC_CACHE:
        _NC_CACHE[key2] = build_gat(plan_pp, tuple(c1['widths']), l3=False)
    nc2 = _NC_CACHE[key2]

    # ---- L1 ----
    x_pad_bf = pre['x_pad_bf']
    x_pad_f = np.asarray(x_pad_bf, np.float32)
    lamL0 = lam_of(x_pad_f[:, :64], c0['Wlam_l'], c0['blam_l'])
    lamR0 = lam_of(x_pad_f[:, :64], c0['Wlam_r'], c0['blam_r'])
    ex1 = [dict(xTs=_xts_slice(x_pad_f[:, :64], c, span))
           for c in range(N_CORES)]
    im1 = layer_in_maps(plan_pp, str_pp, x_pad_bf, c0['Wl'], c0['Wr'],
                        c0['bl'], c0['br'], c0['bprime'], lamL0, lamR0, ex1)
    r1 = run_fn(nc1, im1)
    p0 = np.concatenate([np.asarray(r1[c]['p_out'], np.float32)
                         for c in range(N_CORES)], 0)
    p0[100000:] = 0.0
    p0_bf = to_bf(p0)

    # ---- L2 ---- (same compiled program)
    def rowfix0(W):
        return input_fixup(W, c0['perm'], c0['A'])
    lamL1 = lam_of(p0, rowfix0(c1['Wlam_l']), c1['blam_l'])
    lamR1 = lam_of(p0, rowfix0(c1['Wlam_r']), c1['blam_r'])
    ex2 = [dict(xTs=_xts_slice(p0, c, span)) for c in range(N_CORES)]
    im2 = layer_in_maps(plan_pp, str_pp, p0_bf, rowfix0(c1['Wl']),
                        rowfix0(c1['Wr']), c1['bl'], c1['br'], c1['bprime'],
                        lamL1, lamR1, ex2)
    r2 = run_fn(nc2, im2)
    p1 = np.concatenate([np.asarray(r2[c]['p_out'], np.float32)
                         for c in range(N_CORES)], 0)
    p1[100000:] = 0.0
    p1_bf = to_bf(p1)

    # ---- L3 ----
    def rowfix1(W):
        return input_fixup(W, c1['perm'], c1['A'])
    key3 = ('l3', plan_pf.T, tuple(ct['widths']))
    if key3 not in _NC_CACHE:
        _NC_CACHE[key3] = build_gat(plan_pf, tuple(ct['widths']), l3=True)
    nc3 = _NC_CACHE[key3]
    x_fp_bf = pre['x_fp_bf']
    x_fp_f = np.asarray(x_fp_bf, np.float32)
    lamLt = lam_of(p1, rowfix1(ct['Wlam_l']), ct['blam_l'])
    lamRt = lam_of(x_fp_f, ct['Wlam_r'], ct['blam_r'])
    mw1f = input_fixup(f['m_w1'], ct['perm'], ct['A'])
    col = lambda v: np.ascontiguousarray(
        np.asarray(v, np.float32).reshape(-1, 1))
    ex3 = []
    for c in range(N_CORES):
        ex3.append(dict(
            xTs=_xts_slice(x_fp_f, c, fspan),
            fT=to_bf(x_fp_f[c * fspan:(c + 1) * fspan].T),
            mw1=to_bf(mw1f), mw2=to_bf(f['m_w2']), mw3=to_bf(f['m_w3']),
            nsw=to_bf(f['nm_sw']), nbw=to_bf(f['nm_bw']),
            ncw=to_bf(f['nm_cw']), nlw=to_bf(f['nm_lw']),
            mb1=col(f['m_b1']), mb2=col(f['m_b2']), mb3=col(f['m_b3']),
            nsb=col(f['nm_sb']), nbb=col(f['nm_bb']), ncb=col(f['nm_cb']),
            nlb=col(f['nm_lb'])))
    im3 = layer_in_maps(plan_pf, str_pf, p1_bf, rowfix1(ct['Wl']), ct['Wr'],
                        ct['bl'], ct['br'], ct['bprime'], lamLt, lamRt, ex3)
    r3 = run_fn(nc3, im3)
    out = np.concatenate([np.asarray(r3[c]['out'], np.float32)[0]
                          for c in range(N_CORES)])
    return out[:20000].reshape(20000, 1).astype(np.float32)


# ---------------------------------------------------------------- kernel API
def kernel(**inputs):
    """Self-contained entry: full inputs -> full [20000, 1] float32 output."""
    return run_model(inputs)
